# revision 1
# baseline (speedup 1.0000x reference)
"""Trainium2 Bass kernel for nn_EDTransformer (encoder-decoder transformer).

Sharding: 8 cores = 4 batch items x 2 sequence halves.
 - Each core owns (item b, half h): computes Q/scores/AV/Wo/MLP/LN for its
   256 local positions, K/V redundantly for the full 512 positions.
 - One 2-core AllGather of fp16 activations per layer (pairs share an item).
 - Unembedding sharded over vocab (4000 rows/core, 32 M-tiles of 125),
   softmax denominator via one 8-core AllReduce of (4,512) partial sums.
Dtypes: fp16 matmul operands (weights pre-transposed+cast on host),
 fp32 PSUM accumulation, fp32 residual stream + LN stats, fp32 output.
"""
import os
import sys

sys.path.insert(0, '/opt/trn_rl_repo')
import numpy as np

import concourse.bacc as bacc
import concourse.tile as tile
import concourse.mybir as mybir
from concourse.bass_utils import run_bass_kernel_spmd

DT = mybir.dt
F16 = DT.float16
F32 = DT.float32

N_CORES = 8
P = 128
DE = 1024          # model dim        (8 ptiles)
KO = DE // P       # 8
DMLP = 4096        # mlp dim          (32 ptiles)
MO = DMLP // P     # 32
H = 16             # heads
DA = 64            # attn dim per head
L = 512            # sequence length
LL = 256           # local positions per core
NV = 32000
UC = 500           # unembed vocab chunk (moving N)
UNC = NV // UC     # 64 chunks
LENC = 2
LDEC = 2
EPS = 1e-5

PAIR_GROUPS = [[0, 1], [2, 3], [4, 5], [6, 7]]
ALL_GROUP = [list(range(N_CORES))]

_CACHE = {}


# ----------------------------------------------------------------------------
# device program
# ----------------------------------------------------------------------------

def _attn(nc, tc, pools, Eres32, qin16, kvin16, wq_d, wk_d, wv_d, wo_d,
          mask, name):
    """One multi-head attention block; accumulates Wo output into Eres32.

    qin16  : [128, KO, LL] fp16  local stream (query input)
    kvin16 : [128, KO, L]  fp16  full-sequence stream (key/value input)
    wq_d/wk_d/wv_d/wo_d: dram APs [128, KO, 1024] (pre-transposed; wq scaled)
    mask   : [128, 4, LL] fp16 sbuf tile or None
    """
    sb = pools['att']
    p256 = pools['p256']
    p512 = pools['p512']
    ones = pools['ones']
    KT = L // P  # 4 kz tiles

    # q: [128(2h x 64a), pr, LL]
    q16 = sb.tile([P, KO, LL], F16, tag='q16')
    for pr in range(KO):
        wt = pools['wqp'].tile([P, KO, P], F16, tag='wqt')
        nc.sync.dma_start(wt[:], wq_d[:, :, pr * P:(pr + 1) * P])
        ps = p256.tile([P, LL], F32, tag='p256')
        for k in range(KO):
            nc.tensor.matmul(ps[:], wt[:, k, :],
                             qin16[:, k, :], start=(k == 0), stop=(k == KO - 1))
        nc.vector.tensor_copy(q16[:, pr, :], ps[:])
    # k: [128(2h x 64a), pr, L]
    k16 = sb.tile([P, KO, L], F16, tag='k16')
    for pr in range(KO):
        wt = pools['wkp'].tile([P, KO, P], F16, tag='wkt')
        nc.sync.dma_start(wt[:], wk_d[:, :, pr * P:(pr + 1) * P])
        ps = p512.tile([P, L], F32, tag='p512')
        for k in range(KO):
            nc.tensor.matmul(ps[:], wt[:, k, :],
                             kvin16[:, k, :], start=(k == 0), stop=(k == KO - 1))
        nc.vector.tensor_copy(k16[:, pr, :], ps[:])
    # vT: [128(kz), kt, 1024(h*64+o)]
    vt16 = sb.tile([P, KT, H * DA], F16, tag='vt16')
    for nch in range(2):
        wt = pools['wvp'].tile([P, KO, 512], F16, tag='wvt')
        nc.sync.dma_start(wt[:], wv_d[:, :, nch * 512:(nch + 1) * 512])
        for kt in range(KT):
            ps = p512.tile([P, 512], F32, tag='p512')
            for k in range(KO):
                nc.tensor.matmul(ps[:], kvin16[:, k, kt * P:(kt + 1) * P],
                                 wt[:, k, :],
                                 start=(k == 0), stop=(k == KO - 1))
            nc.vector.tensor_copy(vt16[:, kt, nch * 512:(nch + 1) * 512], ps[:])

    # scores -> exp -> (mask) ; exp16: [128(kz), h, kt, LL]
    exp16 = sb.tile([P, H, KT, LL], F16, tag='exp16')
    for h in range(H):
        pr, hp = h // 2, (h % 2) * DA
        for kt in range(KT):
            ps = p256.tile([P, LL], F32, tag='p256')
            nc.tensor.matmul(ps[:], k16[hp:hp + DA, pr, kt * P:(kt + 1) * P],
                             q16[hp:hp + DA, pr, :], start=True, stop=True)
            nc.scalar.activation(exp16[:, h, kt, :], ps[:],
                                 mybir.ActivationFunctionType.Exp)
    if mask is not None:
        for kt in range(KT):
            nc.vector.tensor_tensor(
                exp16[:, :, kt, :], exp16[:, :, kt, :],
                mask[:, kt, None, :].to_broadcast((P, H, LL)),
                mybir.AluOpType.mult)

    # AV with 2-head column packing + per-head normalize.
    # deno: ones-matmul with M=128 -> every partition row holds the colsums.
    y16 = sb.tile([P, KO, LL], F16, tag='y16')
    for pr in range(KO):
        hA, hB = 2 * pr, 2 * pr + 1
        pd = p512.tile([P, 2 * LL], F32, tag='p512')
        for kt in range(KT):
            nc.tensor.matmul(pd[:], ones[:, :],
                             exp16[:, hA:hB + 1, kt, :],
                             start=(kt == 0), stop=(kt == KT - 1))
        ysc = sb.tile([P, 2, LL], F32, tag='ysc')
        nc.vector.reciprocal(ysc[:], pd[:])
        ps = p256.tile([P, LL], F32, tag='p256')
        for kt in range(KT):
            nc.tensor.matmul(ps[:DA, :], vt16[:, kt, hA * DA:(hA + 1) * DA],
                             exp16[:, hA, kt, :], start=(kt == 0),
                             stop=(kt == KT - 1), tile_position=(0, 0))
            nc.tensor.matmul(ps[DA:, :], vt16[:, kt, hB * DA:(hB + 1) * DA],
                             exp16[:, hB, kt, :], start=(kt == 0),
                             stop=(kt == KT - 1), tile_position=(0, DA))
        nc.vector.tensor_tensor(y16[:DA, pr, :], ps[:DA, :], ysc[:DA, 0, :],
                                mybir.AluOpType.mult)
        nc.vector.tensor_tensor(y16[DA:, pr, :], ps[DA:, :], ysc[DA:, 1, :],
                                mybir.AluOpType.mult)

    # Wo -> accumulate into residual
    for dt in range(KO):
        wt = pools['wop'].tile([P, KO, P], F16, tag='wot')
        nc.sync.dma_start(wt[:], wo_d[:, :, dt * P:(dt + 1) * P])
        ps = p256.tile([P, LL], F32, tag='p256')
        for k in range(KO):
            nc.tensor.matmul(ps[:], wt[:, k, :],
                             y16[:, k, :], start=(k == 0), stop=(k == KO - 1))
        nc.vector.tensor_tensor(Eres32[:, dt, :], Eres32[:, dt, :], ps[:],
                                mybir.AluOpType.add)
    tp = pools.get('tapfn')
    if tp:
        tp(f'{name}_q', q16); tp(f'{name}_k', k16); tp(f'{name}_vt', vt16)
        tp(f'{name}_exp', exp16); tp(f'{name}_y', y16)


def _mlp(nc, tc, pools, Eres32, ein16, w1_dram, w2_dram, name):
    p256 = pools['p256']
    h16 = pools['mlp'].tile([P, MO, LL], F16, tag='h16')
    for mt in range(MO):
        w1t = pools['w1p'].tile([P, KO, P], F16, tag='w1t')
        nc.sync.dma_start(w1t[:], w1_dram[:, :, mt * P:(mt + 1) * P])
        ps = p256.tile([P, LL], F32, tag='p256')
        for k in range(KO):
            nc.tensor.matmul(ps[:], w1t[:, k, :], ein16[:, k, :],
                             start=(k == 0), stop=(k == KO - 1))
        nc.scalar.activation(h16[:, mt, :], ps[:],
                             mybir.ActivationFunctionType.Relu)
    for dt in range(KO):
        w2t = pools['w2p'].tile([P, MO, P], F16, tag='w2t')
        nc.sync.dma_start(w2t[:], w2_dram[:, :, dt * P:(dt + 1) * P])
        ps = p256.tile([P, LL], F32, tag='p256')
        for k in range(MO):
            nc.tensor.matmul(ps[:], w2t[:, k, :], h16[:, k, :],
                             start=(k == 0), stop=(k == MO - 1))
        nc.vector.tensor_tensor(Eres32[:, dt, :], Eres32[:, dt, :], ps[:],
                                mybir.AluOpType.add)


def _ln(nc, tc, pools, Eres32, e16out, name):
    """In-place layernorm over features; writes fp16 copy to e16out."""
    p256 = pools['p256']
    ones = pools['ones']
    stat = pools['stat']

    e16pre = pools['lnp'].tile([P, KO, LL], F16, tag='e16pre')
    nc.vector.tensor_copy(e16pre[:], Eres32[:])
    sq16 = pools['lnp'].tile([P, KO, LL], F16, tag='sq16')
    nc.vector.tensor_tensor(sq16[:], e16pre[:], e16pre[:],
                            mybir.AluOpType.mult)
    # sums with M=128 ones -> replicated rows; stats stay [128, LL]
    pss = p256.tile([P, LL], F32, tag='p256')
    psq = p256.tile([P, LL], F32, tag='p256')
    for k in range(KO):
        nc.tensor.matmul(pss[:], ones[:, :], e16pre[:, k, :],
                         start=(k == 0), stop=(k == KO - 1))
    for k in range(KO):
        nc.tensor.matmul(psq[:], ones[:, :], sq16[:, k, :],
                         start=(k == 0), stop=(k == KO - 1))
    mean = stat.tile([P, LL], F32, tag='mean')
    nc.vector.tensor_scalar_mul(mean[:], pss[:], 1.0 / DE)
    var = stat.tile([P, LL], F32, tag='var')
    nc.vector.tensor_scalar_mul(var[:], psq[:], 1.0 / DE)
    msq = stat.tile([P, LL], F32, tag='msq')
    nc.vector.tensor_tensor(msq[:], mean[:], mean[:], mybir.AluOpType.mult)
    nc.vector.tensor_tensor(var[:], var[:], msq[:], mybir.AluOpType.subtract)
    nc.vector.tensor_scalar_mul(var[:], var[:], float(DE) / (DE - 1))
    std = stat.tile([P, LL], F32, tag='std')
    nc.scalar.activation(std[:], var[:], mybir.ActivationFunctionType.Sqrt,
                         bias=pools['eps128'])
    inv = stat.tile([P, LL], F32, tag='inv')
    nc.vector.reciprocal(inv[:], std[:])
    negms = stat.tile([P, LL], F32, tag='negms')
    nc.vector.tensor_tensor(negms[:], mean[:], inv[:], mybir.AluOpType.mult)
    nc.vector.tensor_scalar_mul(negms[:], negms[:], -1.0)
    nc.vector.tensor_tensor(
        Eres32[:], Eres32[:],
        inv[:, None, :].to_broadcast((P, KO, LL)), mybir.AluOpType.mult)
    nc.vector.tensor_tensor(
        Eres32[:], Eres32[:],
        negms[:, None, :].to_broadcast((P, KO, LL)), mybir.AluOpType.add)
    nc.vector.tensor_copy(e16out[:], Eres32[:])
    tp = pools.get('tapfn')
    if tp:
        tp(f'{name}_out', Eres32)


def _allgather_pair(nc, tc, pools, e16loc, full16, agin, agout, tag):
    """e16loc [128, KO, LL] -> pair AllGather -> full16 [128, KO, L]."""
    nc.gpsimd.dma_start(agin[:], e16loc[:])
    nc.gpsimd.collective_compute(
        "AllGather", mybir.AluOpType.bypass,
        ins=[agin[:]], outs=[agout[:]],
        replica_groups=PAIR_GROUPS)
    nc.gpsimd.dma_start(
        full16[:].rearrange('ki ko (r p) -> ki ko r p', r=2),
        agout[:].rearrange('r ki ko p -> ki ko r p'))


def build_program(taps=()):
    taps = set(taps)
    nc = bacc.Bacc("TRN2", target_bir_lowering=False, debug=False,
                   num_devices=N_CORES)

    # ---- dram inputs ----
    din = {}
    def dram_in(nm, shape, dt=F16):
        din[nm] = nc.dram_tensor(nm, list(shape), dt, kind="ExternalInput")
        return din[nm]

    z0f = dram_in('z0_full16', [P, KO, L])
    x0f = dram_in('x0_full16', [P, KO, L])
    z0l32 = dram_in('z0_loc32', [P, KO, LL], F32)
    x0l32 = dram_in('x0_loc32', [P, KO, LL], F32)
    z0l16 = dram_in('z0_loc16', [P, KO, LL])
    x0l16 = dram_in('x0_loc16', [P, KO, LL])
    mask_self = dram_in('mask_self', [P, 4, LL])
    for pfx, nl in (('enc', LENC), ('dec', LDEC)):
        for w in ('wqT', 'wkT', 'wvT', 'woT'):
            dram_in(f'{pfx}_{w}', [nl, P, KO, DE])
        dram_in(f'{pfx}_w1T', [nl, P, KO, DMLP])
        dram_in(f'{pfx}_w2T', [nl, P, MO, DE])
    wuT = dram_in('wuT', [P, KO, NV])

    # output: [pt, ki, vocab] fp16, positions = h*256 + pt*128 + ki
    outp = nc.dram_tensor('outp', [2, P, NV], mybir.dt.float16,
                          kind="ExternalOutput")

    # internal dram for pair collectives
    agin = nc.dram_tensor('agin', [P, KO, LL], F16)
    agout = nc.dram_tensor('agout', [2, P, KO, LL], F16)


    import contextlib
    with tile.TileContext(nc) as tc, contextlib.ExitStack() as octx:
        const = octx.enter_context(tc.tile_pool(name='const', bufs=1))
        ones = const.tile([P, P], F16)
        nc.vector.memset(ones[:], 1.0)
        eps1 = const.tile([1, 1], F32)
        nc.vector.memset(eps1[:], EPS)
        eps128 = const.tile([P, 1], F32)
        nc.vector.memset(eps128[:], EPS)
        xu = const.tile([P, KO, LL], F16)
        msk = const.tile([P, 4, LL], F16)
        nc.sync.dma_start(msk[:], mask_self[:])

        # ================= layer phase =================
        with contextlib.ExitStack() as ctx:
            stream = ctx.enter_context(tc.tile_pool(name='stream', bufs=1))
            att = ctx.enter_context(tc.tile_pool(name='att', bufs=1))
            mlpp = ctx.enter_context(tc.tile_pool(name='mlpp', bufs=1))
            lnp = ctx.enter_context(tc.tile_pool(name='lnp', bufs=1))
            stat = ctx.enter_context(tc.tile_pool(name='stat', bufs=1))
            wqp = ctx.enter_context(tc.tile_pool(name='wqp', bufs=3))
            wkp = ctx.enter_context(tc.tile_pool(name='wkp', bufs=3))
            wvp = ctx.enter_context(tc.tile_pool(name='wvp', bufs=2))
            wop = ctx.enter_context(tc.tile_pool(name='wop', bufs=3))
            w1p = ctx.enter_context(tc.tile_pool(name='w1p', bufs=4))
            w2p = ctx.enter_context(tc.tile_pool(name='w2p', bufs=2))
            p256 = ctx.enter_context(tc.tile_pool(name='p256', bufs=5,
                                                  space='PSUM'))
            p512 = ctx.enter_context(tc.tile_pool(name='p512', bufs=3,
                                                  space='PSUM'))

            pools = dict(att=att, mlp=mlpp, lnp=lnp, p256=p256, p512=p512,
                         stat=stat, ones=ones,
                         eps1=eps1, eps128=eps128[:], wqp=wqp, wkp=wkp,
                         wvp=wvp, wop=wop, w1p=w1p, w2p=w2p)

            def tapfn(nm, t):
                if nm not in taps:
                    return
                d = nc.dram_tensor('tap_' + nm, list(t.shape),
                                   t.dtype, kind="ExternalOutput")
                nc.sync.dma_start(d[:], t[:])
            pools['tapfn'] = tapfn

            # ======== encoder ========
            Eres = stream.tile([P, KO, LL], F32, tag='res')
            nc.sync.dma_start(Eres[:], z0l32[:])
            Zfull = stream.tile([P, KO, L], F16, tag='Zfull')
            nc.sync.dma_start(Zfull[:], z0f[:])
            eloc = stream.tile([P, KO, LL], F16, tag='loc_a')
            nc.sync.dma_start(eloc[:], z0l16[:])

            for l in range(LENC):
                _attn(nc, tc, pools, Eres, eloc, Zfull,
                      din['enc_wqT'][l], din['enc_wkT'][l],
                      din['enc_wvT'][l], din['enc_woT'][l], None, f'e{l}a')
                eloc = stream.tile([P, KO, LL], F16, tag='loc_b')
                _ln(nc, tc, pools, Eres, eloc, f'e{l}ln1')
                _mlp(nc, tc, pools, Eres, eloc, din['enc_w1T'][l],
                     din['enc_w2T'][l], f'e{l}m')
                eloc = stream.tile([P, KO, LL], F16, tag='loc_a')
                _ln(nc, tc, pools, Eres, eloc, f'e{l}ln2')
                Zfull = stream.tile([P, KO, L], F16, tag='Zfull')
                _allgather_pair(nc, tc, pools, eloc, Zfull, agin, agout,
                                f'e{l}')


            # ======== decoder ========
            Eres = stream.tile([P, KO, LL], F32, tag='res')
            nc.sync.dma_start(Eres[:], x0l32[:])
            Xfull = stream.tile([P, KO, L], F16, tag='Xfull')
            nc.sync.dma_start(Xfull[:], x0f[:])
            eloc = stream.tile([P, KO, LL], F16, tag='loc_a')
            nc.sync.dma_start(eloc[:], x0l16[:])

            for l in range(LDEC):
                _attn(nc, tc, pools, Eres, eloc, Xfull,
                      din['dec_wqT'][l], din['dec_wkT'][l],
                      din['dec_wvT'][l], din['dec_woT'][l], msk, f'd{l}s')
                eloc = stream.tile([P, KO, LL], F16, tag='loc_b')
                _ln(nc, tc, pools, Eres, eloc, f'd{l}ln1')
                _attn(nc, tc, pools, Eres, eloc, Zfull,
                      din['dec_wqT'][l], din['dec_wkT'][l],
                      din['dec_wvT'][l], din['dec_woT'][l], None, f'd{l}c')
                eloc = stream.tile([P, KO, LL], F16, tag='loc_c')
                _ln(nc, tc, pools, Eres, eloc, f'd{l}ln2')
                _mlp(nc, tc, pools, Eres, eloc, din['dec_w1T'][l],
                     din['dec_w2T'][l], f'd{l}m')
                if l < LDEC - 1:
                    eloc = stream.tile([P, KO, LL], F16, tag='loc_a')
                    _ln(nc, tc, pools, Eres, eloc, f'd{l}ln3')
                    Xfull = stream.tile([P, KO, L], F16, tag='Xfull')
                    _allgather_pair(nc, tc, pools, eloc, Xfull, agin, agout,
                                    f'd{l}')
                else:
                    # final LN writes straight into the outer-scope xu tile
                    _ln(nc, tc, pools, Eres, xu, f'd{l}ln3')

        # ========== unembed phase (position-sharded, no collectives) ========
        with contextlib.ExitStack() as ctx:
            usb = ctx.enter_context(tc.tile_pool(name='usb', bufs=1))
            wup = ctx.enter_context(tc.tile_pool(name='wup', bufs=3))
            up = ctx.enter_context(tc.tile_pool(name='up', bufs=4,
                                                space='PSUM'))
            expu = [usb.tile([P, UNC, UC], F16, tag=f'expu{pt}',
                             name=f'expu{pt}')
                    for pt in range(2)]
            dacc = usb.tile([P, 2, UNC], F32, tag='dacc')
            dma_engs = [nc.sync, nc.gpsimd]
            for c in range(UNC):
                wut = wup.tile([P, KO, UC], F16, tag='wut')
                nc.sync.dma_start(wut[:], wuT[:, :, c * UC:(c + 1) * UC])
                for pt in range(2):
                    ps = up.tile([P, UC], F32, tag='ups', name=f'ups{c}{pt}')
                    for k in range(KO):
                        nc.tensor.matmul(ps[:], xu[:, k, pt * P:(pt + 1) * P],
                                         wut[:, k, :], start=(k == 0),
                                         stop=(k == KO - 1))
                    nc.scalar.activation(expu[pt][:, c, :], ps[:],
                                         mybir.ActivationFunctionType.Exp)
                    nc.vector.tensor_reduce(dacc[:, pt, c:c + 1],
                                            expu[pt][:, c, :],
                                            mybir.AxisListType.X,
                                            mybir.AluOpType.add)
            deno = usb.tile([P, 2], F32, tag='deno')
            nc.vector.tensor_reduce(deno[:], dacc[:], mybir.AxisListType.X,
                                    mybir.AluOpType.add)
            dinv = usb.tile([P, 2], F32, tag='dinv')
            nc.vector.reciprocal(dinv[:], deno[:])
            if 'deno' in taps:
                d = nc.dram_tensor('tap_deno', [P, 2], F32,
                                   kind="ExternalOutput")
                nc.sync.dma_start(d[:], deno[:])
            QN = UNC // 4
            for pt in range(2):
                for q in range(4):
                    seg = expu[pt][:, q * QN:(q + 1) * QN, :]
                    nc.vector.tensor_tensor(
                        seg, seg,
                        dinv[:, pt:pt + 1, None].to_broadcast((P, QN, UC)),
                        mybir.AluOpType.mult)
                    dma_engs[(pt * 4 + q) % 2].dma_start(
                        outp[pt, :, q * QN * UC:(q + 1) * QN * UC], seg)

    nc.compile()
    return nc


# ----------------------------------------------------------------------------
# host-side prep
# ----------------------------------------------------------------------------

def _to_kimaj(a):
    """[K, M] -> [128, K//128, M] with K = ko*128 + ki."""
    K, M = a.shape
    return np.ascontiguousarray(
        a.reshape(K // P, P, M).transpose(1, 0, 2))


def prep_inputs(inputs):
    f = lambda k: np.asarray(inputs[k], dtype=np.float32)
    We, Wp, Wu = f('We'), f('Wp'), f('Wu')
    x = np.asarray(inputs['x']).astype(np.int64)
    z = np.asarray(inputs['z']).astype(np.int64)

    shared = {}
    for pfx, nl in (('enc', LENC), ('dec', LDEC)):
        Wq, Wk, Wv = f(pfx + '_Wq'), f(pfx + '_Wk'), f(pfx + '_Wv')
        Wo, W1, W2 = f(pfx + '_Wo'), f(pfx + '_W1'), f(pfx + '_W2')
        wq, wk, wv, wo, w1, w2 = [], [], [], [], [], []
        for l in range(nl):
            qa = Wq[l].transpose(2, 0, 1).reshape(DE, H * DA) * (DA ** -0.5)
            ka = Wk[l].transpose(2, 0, 1).reshape(DE, H * DA)
            va = Wv[l].transpose(2, 0, 1).reshape(DE, H * DA)
            wq.append(_to_kimaj(qa)); wk.append(_to_kimaj(ka))
            wv.append(_to_kimaj(va))
            wo.append(_to_kimaj(Wo[l].T))
            w1.append(_to_kimaj(W1[l].T))
            w2.append(_to_kimaj(W2[l].T))
        shared[f'{pfx}_wqT'] = np.stack(wq).astype(np.float16)
        shared[f'{pfx}_wkT'] = np.stack(wk).astype(np.float16)
        shared[f'{pfx}_wvT'] = np.stack(wv).astype(np.float16)
        shared[f'{pfx}_woT'] = np.stack(wo).astype(np.float16)
        shared[f'{pfx}_w1T'] = np.stack(w1).astype(np.float16)
        shared[f'{pfx}_w2T'] = np.stack(w2).astype(np.float16)

    shared['wuT'] = _to_kimaj(Wu.T).astype(np.float16)

    pos = Wp[:L]  # [512, 1024]
    in_maps = []
    for c in range(N_CORES):
        b, h = c // 2, c % 2
        m = dict(shared)
        for nm, tok in (('z0', z[b]), ('x0', x[b])):
            E0 = (We[tok] + pos).T.astype(np.float32)      # [1024, 512]
            E0k = E0.reshape(KO, P, L)                     # [ko, ki, p]
            m[nm + '_full16'] = np.ascontiguousarray(
                E0k.transpose(1, 0, 2)).astype(np.float16)
            loc = E0k[:, :, h * LL:(h + 1) * LL].transpose(1, 0, 2)
            m[nm + '_loc32'] = np.ascontiguousarray(loc)
            m[nm + '_loc16'] = np.ascontiguousarray(loc).astype(np.float16)
        kglob = np.arange(L)[:, None]
        qglob = (h * LL + np.arange(LL))[None, :]
        msk = (kglob <= qglob).astype(np.float16)          # [512, 256]
        m['mask_self'] = np.ascontiguousarray(
            msk.reshape(4, P, LL).transpose(1, 0, 2))
        in_maps.append(m)
    return in_maps


def assemble(results):
    """results: per-core dicts with 'outp' [2, 128, NV] fp16."""
    out = np.empty((4, NV, L), dtype=np.float32)
    for c, r in enumerate(results):
        b, h = c // 2, c % 2
        o = r['outp'].reshape(LL, NV)                      # [pos, vocab]
        out[b, :, h * LL:(h + 1) * LL] = o.T.astype(np.float32)
    return out


def run(inputs, trace=False, taps=(), trace_kwargs=None):
    key = ('prog', tuple(sorted(taps)))
    if key not in _CACHE:
        _CACHE[key] = build_program(taps=taps)
    nc = _CACHE[key]
    in_maps = prep_inputs(inputs)
    res = run_bass_kernel_spmd(nc, in_maps, list(range(N_CORES)),
                               trace=trace, **(trace_kwargs or {}))
    return res


def kernel(**inputs):
    res = run(inputs, trace=False)
    return assemble(res.results)



# revision 12
# speedup vs baseline: 1.3181x; 1.3181x over previous
"""Trainium2 Bass kernel for nn_EDTransformer (encoder-decoder transformer).

Sharding: 8 cores = 4 batch items x 2 sequence halves.
 - Each core owns (item b, half h): Q/scores/AV/Wo/MLP/LN for its 256 local
   positions, K/V redundantly for the full 512 positions.
 - One 2-core AllGather of fp8 activations per layer boundary.
 - Unembedding position-sharded (full vocab per core), softmax denominator
   via fused activation accumulate.

Precision plan:
 - Attention (Q/K/V/scores/AV/Wo) in fp8e4 DoubleRow matmuls (2x PE rate).
   Weights scaled x32 on host; softmax normalization folds the scales; the
   residual accumulation lives in PSUM at scale 32768 injected by an
   identity matmul, and the post-LN architecture makes the scale vanish in
   layernorm (scale-invariant).
 - MLP + unembed + LN stats in fp16 with fp32 PSUM accumulation.
"""
import os
import sys

sys.path.insert(0, '/opt/trn_rl_repo')
import numpy as np

import concourse.bacc as bacc
import concourse.tile as tile
import concourse.mybir as mybir
from concourse.bass_utils import run_bass_kernel_spmd

DT = mybir.dt
F16 = DT.float16
F32 = DT.float32
F8 = DT.float8e4
DR = mybir.MatmulPerfMode.DoubleRow
AF = mybir.ActivationFunctionType
ALU = mybir.AluOpType

N_CORES = 8
P = 128
DE = 1024          # model dim        (8 ptiles)
KO = DE // P       # 8
DMLP = 4096        # mlp dim          (32 ptiles)
MO = DMLP // P     # 32
H = 16             # heads
DA = 64            # attn dim per head
L = 512            # sequence length
LL = 256           # local positions per core
NV = 32000
UC = 500           # unembed vocab chunk (moving N)
UNC = NV // UC     # 64 chunks
LENC = 2
LDEC = 2
EPS = 1e-5

WS = 32.0               # weight scale for fp8
# y8 = WS * y keeps |y8| <= max|v8| (attention out is convex comb of v).
SRES = float(WS * WS)   # 1024: attn residual psum scale (wo8 @ y8)
EXPS = 1.0 / (WS * WS * 8.0)  # exp activation scale (folds q*k scale + sqrt(da))

PAIR_GROUPS = [[0, 1], [2, 3], [4, 5], [6, 7]]

_CACHE = {}


# ----------------------------------------------------------------------------
# device program
# ----------------------------------------------------------------------------

def _attn(nc, tc, pools, pres, e16res, qin8, kv8, wq_t, wk_t, wv_t, wo_t,
          mask8, name):
    """One multi-head attention block. Leaves pres [128, 8, 256] f32 psum
    holding SRES * (attn_out + residual).

    qin8   : [128, KO, LL] fp8  local stream (query input)
    kv8    : [128, KO, L]  fp8  full-sequence stream (key/value input)
    wq_t..wo_t : SBUF fp8 weight tiles [128, KO, 1024] (x32 scaled,
                 q/k with 32-split head packing)
    mask8  : [128, 4, LL] fp8 sbuf tile or None
    """
    sb = pools['att']
    pA = pools['pA']
    KT = L // P  # 4 kz tiles

    # ---- residual identity injection (also zeroes pres via start=True on
    # even dt: the hw zero-region is 2KB = a dt pair, so even-dt start
    # pre-zeroes its odd partner before that partner's start=False add).
    for dt in range(KO):
        nc.tensor.matmul(pres[:, dt, :], pools['diag32k'][:, :],
                         e16res[:, dt, :], start=(dt % 2 == 0), stop=False,
                         skip_group_check=True)

    # ---- q: [128(4h x 32a, split-packed), pr, LL] fp8
    q8 = sb.tile([P, KO, LL], F8, tag='q8')
    for pr in range(KO):
        ps = pA.tile([P, L], F32, tag='pA')
        for j in range(KO // 2):
            nc.tensor.matmul(ps[:, :LL], wq_t[:, 2 * j:2 * j + 2,
                                              pr * P:(pr + 1) * P],
                             qin8[:, 2 * j:2 * j + 2, :],
                             start=(j == 0), stop=(j == 3), perf_mode=DR)
        nc.scalar.activation(q8[:, pr, :], ps[:, :LL], AF.Copy)
    # ---- k: [128, pr, L] fp8
    k8 = sb.tile([P, KO, L], F8, tag='k8')
    for pr in range(KO):
        ps = pA.tile([P, L], F32, tag='pA')
        for j in range(KO // 2):
            nc.tensor.matmul(ps[:], wk_t[:, 2 * j:2 * j + 2,
                                         pr * P:(pr + 1) * P],
                             kv8[:, 2 * j:2 * j + 2, :],
                             start=(j == 0), stop=(j == 3), perf_mode=DR)
        nc.scalar.activation(k8[:, pr, :], ps[:], AF.Copy)
    # ---- vT: [128(kz), kt, 1024(h*64)] fp8
    vt8 = sb.tile([P, KT, H * DA], F8, tag='vt8')
    for kt in range(KT):
        for nch in range(2):
            ps = pA.tile([P, L], F32, tag='pA')
            for j in range(KO // 2):
                nc.tensor.matmul(
                    ps[:], kv8[:, 2 * j:2 * j + 2, kt * P:(kt + 1) * P],
                    wv_t[:, 2 * j:2 * j + 2, nch * 512:(nch + 1) * 512],
                    start=(j == 0), stop=(j == 3), perf_mode=DR)
            nc.vector.tensor_copy(vt8[:, kt, nch * 512:(nch + 1) * 512],
                                  ps[:])

    # ---- scores -> exp (fp8), per head, kt-pair granularity
    # exp8 flat layout [128(kz), 4(kt), 16*256 (h-major)]
    exp8 = sb.tile([P, KT, H * LL], F8, tag='exp8')
    for h in range(H):
        pr, hp = h // 2, (h % 2) * DA
        for t in range(2):
            ps = pA.tile([P, L], F32, tag='pA')
            for kt in (2 * t, 2 * t + 1):
                nc.tensor.matmul(
                    ps[:, (kt - 2 * t) * LL:(kt - 2 * t + 1) * LL],
                    k8[hp:hp + DA, pr, kt * P:(kt + 1) * P],
                    q8[hp:hp + DA, pr, :],
                    start=True, stop=True)
            nc.scalar.activation(
                exp8[:, 2 * t:2 * t + 2, h * LL:(h + 1) * LL],
                ps[:].rearrange('p (a b) -> p a b', a=2),
                AF.Exp, scale=EXPS)
        if mask8 is not None:
            nc.vector.tensor_tensor(
                exp8[:, :, h * LL:(h + 1) * LL],
                exp8[:, :, h * LL:(h + 1) * LL],
                mask8[:], ALU.mult)

    # ---- deno + AV + normalize per head pair
    y8 = sb.tile([P, KO, LL], F8, tag='y8')
    for p in range(KO):
        hA = 2 * p
        pd = pA.tile([P, L], F32, tag='pA')
        for t in range(2):
            nc.tensor.matmul(pd[:], pools['ones8d'][:, :, :],
                             exp8[:, 2 * t:2 * t + 2,
                                  hA * LL:hA * LL + 2 * LL],
                             start=(t == 0), stop=(t == 1), perf_mode=DR)
        ysc = pools['yscp'].tile([P, 2 * LL], F32, tag='ysc')
        nc.vector.reciprocal_approx_fast(ysc[:], pd[:])
        # DoubleRow disallows column tiling: use full 128-col lhsT windows;
        # head A lands in out rows 0:64 (cols hA*64..), head B in rows
        # 64:128 (window shifted down 64 cols) — unread rows are garbage.
        pavA = pA.tile([P, L], F32, tag='pA')
        pavB = pA.tile([P, L], F32, tag='pA')
        for t in range(2):
            nc.tensor.matmul(
                pavA[:, :LL],
                vt8[:, 2 * t:2 * t + 2, hA * DA:(hA + 2) * DA],
                exp8[:, 2 * t:2 * t + 2, hA * LL:(hA + 1) * LL],
                start=(t == 0), stop=(t == 1), perf_mode=DR)
        for t in range(2):
            nc.tensor.matmul(
                pavB[:, :LL],
                vt8[:, 2 * t:2 * t + 2, hA * DA:(hA + 2) * DA],
                exp8[:, 2 * t:2 * t + 2, (hA + 1) * LL:(hA + 2) * LL],
                start=(t == 0), stop=(t == 1), perf_mode=DR)
        nc.vector.tensor_tensor(y8[:DA, p, :], pavA[:DA, :LL],
                                ysc[:DA, 0:LL], ALU.mult)
        nc.vector.tensor_tensor(y8[DA:, p, :], pavB[DA:, :LL],
                                ysc[DA:, LL:2 * LL], ALU.mult)

    # ---- Wo accumulate into pres (start=False: id-matmul zeroed/seeded)
    for dt in range(KO):
        for j in range(KO // 2):
            nc.tensor.matmul(pres[:, dt, :],
                             wo_t[:, 2 * j:2 * j + 2, dt * P:(dt + 1) * P],
                             y8[:, 2 * j:2 * j + 2, :],
                             start=False, stop=(j == 3),
                             perf_mode=DR, skip_group_check=True)
    tp = pools.get('tapfn')
    if tp:
        tp(f'{name}_q', q8); tp(f'{name}_k', k8); tp(f'{name}_vt', vt8)
        tp(f'{name}_exp', exp8); tp(f'{name}_y', y8)


def _mlp(nc, tc, pools, pres, e16res, w1_dram, w2_dram, qs, name):
    """MLP block (fp16). Leaves pres holding 1.0 * (mlp_out + residual)."""
    pA = pools['pA']
    for dt in range(KO):
        nc.tensor.matmul(pres[:, dt, :], pools['diag1'][:, :],
                         e16res[:, dt, :], start=(dt % 2 == 0), stop=False,
                         skip_group_check=True)
    h16 = pools['mlp'].tile([P, MO, LL], F16, tag='h16')
    for c in range(8):
        w1t = pools['w1p'].tile([P, KO, 4 * P], F16, tag='w1t')
        qs[c % 2].dma_start(w1t[:], w1_dram[:, :, c * 512:(c + 1) * 512])
        for m2 in range(2):   # 2 mt per psum tile
            ps = pA.tile([P, L], F32, tag='pA')
            for mi in range(2):
                for k in range(KO):
                    nc.tensor.matmul(ps[:, mi * LL:(mi + 1) * LL],
                                     w1t[:, k, (2 * m2 + mi) * P:
                                         (2 * m2 + mi + 1) * P],
                                     e16res[:, k, :],
                                     start=(k == 0), stop=(k == KO - 1))
            nc.scalar.activation(h16[:, 4 * c + 2 * m2:4 * c + 2 * m2 + 2, :],
                                 ps[:].rearrange('p (a b) -> p a b', a=2),
                                 AF.Relu)
    for c in range(8):
        w2t = pools['w2p'].tile([P, 4, KO * P], F16, tag='w2t')
        qs[c % 2].dma_start(w2t[:], w2_dram[:, c * 4:(c + 1) * 4, :])
        for j in range(4):
            for dt in range(KO):
                nc.tensor.matmul(pres[:, dt, :],
                                 w2t[:, j, dt * P:(dt + 1) * P],
                                 h16[:, c * 4 + j, :],
                                 start=False,
                                 stop=(c == 7 and j == 3),
                                 skip_group_check=True)


def _ln(nc, tc, pools, pres, sres_inv, e16out, e8out, name):
    """Layernorm from psum pres (scale 1/sres_inv) -> fp16 (+fp8) stream."""
    pA = pools['pA']
    lnp = pools['lnp']
    stat = pools['stat']
    pre16 = lnp.tile([P, KO, LL], F16, tag='pre16')
    for dt in range(KO):
        nc.scalar.activation(pre16[:, dt, :], pres[:, dt, :], AF.Copy,
                             scale=sres_inv)
    sq16 = lnp.tile([P, KO, LL], F16, tag='sq16')
    nc.vector.tensor_tensor(sq16[:], pre16[:], pre16[:], ALU.mult)
    pss = pA.tile([P, L], F32, tag='pA')
    for k in range(KO):
        nc.tensor.matmul(pss[:, :LL], pools['ones16'][:, :], pre16[:, k, :],
                         start=(k == 0), stop=(k == KO - 1))
    psq = pA.tile([P, L], F32, tag='pA')
    for k in range(KO):
        nc.tensor.matmul(psq[:, :LL], pools['ones16'][:, :], sq16[:, k, :],
                         start=(k == 0), stop=(k == KO - 1))
    # var = Q/1023 - S^2/(1024*1023); inv = 1/sqrt(var+eps); nm = -S/1024*inv
    s2 = stat.tile([P, LL], F32, tag='s2')
    nc.scalar.activation(s2[:], pss[:, :LL], AF.Square)
    var = stat.tile([P, LL], F32, tag='var')
    nc.vector.tensor_scalar(var[:], s2[:], 1.0 / (1024.0 * 1023.0), None,
                            ALU.mult)
    q1 = stat.tile([P, LL], F32, tag='q1')
    nc.vector.tensor_scalar(q1[:], psq[:, :LL], 1.0 / 1023.0, None, ALU.mult)
    nc.vector.tensor_tensor(var[:], q1[:], var[:], ALU.subtract)
    std = stat.tile([P, LL], F32, tag='std')
    nc.scalar.activation(std[:], var[:], AF.Sqrt, bias=pools['eps128'])
    inv = stat.tile([P, LL], F32, tag='inv')
    nc.vector.reciprocal_approx_fast(inv[:], std[:])
    nm = stat.tile([P, LL], F32, tag='nm')
    nc.vector.tensor_tensor(nm[:], pss[:, :LL], inv[:], ALU.mult)
    nc.vector.tensor_scalar(nm[:], nm[:], -1.0 / 1024.0, None, ALU.mult)
    nc.vector.tensor_tensor(
        e16out[:], pre16[:],
        inv[:, None, :].to_broadcast((P, KO, LL)), ALU.mult)
    nc.vector.tensor_tensor(
        e16out[:], e16out[:],
        nm[:, None, :].to_broadcast((P, KO, LL)), ALU.add)
    if e8out is not None:
        nc.vector.tensor_copy(e8out[:], e16out[:])
    tp = pools.get('tapfn')
    if tp:
        tp(f'{name}_out', e16out)


def _allgather_pair(nc, tc, pools, e8loc, full8, agin, agout, tag):
    """e8loc [128, KO, LL] fp8 -> pair AllGather -> full8 [128, KO, L]."""
    nc.gpsimd.dma_start(agin[:], e8loc[:])
    nc.gpsimd.collective_compute(
        "AllGather", ALU.bypass,
        ins=[agin[:]], outs=[agout[:]],
        replica_groups=PAIR_GROUPS)
    nc.gpsimd.dma_start(
        full8[:].rearrange('ki ko (r p) -> ki ko r p', r=2),
        agout[:].rearrange('r ki ko p -> ki ko r p'))


def build_program(taps=()):
    taps = set(taps)
    nc = bacc.Bacc("TRN2", target_bir_lowering=False, debug=False,
                   num_devices=N_CORES)

    # ---- dram inputs ----
    din = {}
    def dram_in(nm, shape, dt=F8):
        din[nm] = nc.dram_tensor(nm, list(shape), dt, kind="ExternalInput")
        return din[nm]

    z0f8 = dram_in('z0_full8', [P, KO, L])
    x0f8 = dram_in('x0_full8', [P, KO, L])
    z0l8 = dram_in('z0_loc8', [P, KO, LL])
    x0l8 = dram_in('x0_loc8', [P, KO, LL])
    z0l16 = dram_in('z0_loc16', [P, KO, LL], F16)
    x0l16 = dram_in('x0_loc16', [P, KO, LL], F16)
    mask_self = dram_in('mask_self8', [P, 4, LL])
    diag32k_d = dram_in('diag32k', [P, P], F16)
    diag1_d = dram_in('diag1', [P, P], F16)
    for pfx, nl in (('enc', LENC), ('dec', LDEC)):
        for w in ('wq8', 'wk8', 'wv8', 'wo8'):
            dram_in(f'{pfx}_{w}', [nl, P, KO, DE], F8)
        dram_in(f'{pfx}_w1T', [nl, P, KO, DMLP], F16)
        dram_in(f'{pfx}_w2T', [nl, P, MO, DE], F16)
    wuT = dram_in('wuT', [P, KO, NV], F16)

    # output: [pt, ki, vocab] fp16, positions = h*256 + pt*128 + ki
    outp = nc.dram_tensor('outp', [2, P, NV], F16, kind="ExternalOutput")

    # internal dram for pair collectives
    agin = nc.dram_tensor('agin', [P, KO, LL], F8)
    agout = nc.dram_tensor('agout', [2, P, KO, LL], F8)

    import contextlib
    with tile.TileContext(nc) as tc, contextlib.ExitStack() as octx:
        const = octx.enter_context(tc.tile_pool(name='const', bufs=1))
        ones16 = const.tile([P, P], F16)
        nc.vector.memset(ones16[:], 1.0)
        ones8d = const.tile([P, 2, P], F8)
        nc.vector.memset(ones8d[:], 1.0)
        eps128 = const.tile([P, 1], F32)
        nc.vector.memset(eps128[:], EPS)
        diag32k = const.tile([P, P], F16)
        nc.sync.dma_start(diag32k[:], diag32k_d[:])
        diag1 = const.tile([P, P], F16)
        nc.scalar.dma_start(diag1[:], diag1_d[:])
        msk8 = const.tile([P, 4, LL], F8)
        nc.sync.dma_start(msk8[:], mask_self[:])
        xu = const.tile([P, KO, LL], F16)

        # ================= layer phase =================
        with contextlib.ExitStack() as ctx:
            stream = ctx.enter_context(tc.tile_pool(name='stream', bufs=1))
            att = ctx.enter_context(tc.tile_pool(name='att', bufs=1))
            mlpp = ctx.enter_context(tc.tile_pool(name='mlpp', bufs=1))
            lnp = ctx.enter_context(tc.tile_pool(name='lnp', bufs=1))
            stat = ctx.enter_context(tc.tile_pool(name='stat', bufs=1))
            yscp = ctx.enter_context(tc.tile_pool(name='yscp', bufs=2))
            watp = ctx.enter_context(tc.tile_pool(name='watp', bufs=2))
            w1p = ctx.enter_context(tc.tile_pool(name='w1p', bufs=2))
            w2p = ctx.enter_context(tc.tile_pool(name='w2p', bufs=2))
            pA = ctx.enter_context(tc.tile_pool(name='pA', bufs=4,
                                                space='PSUM'))
            presp = ctx.enter_context(tc.tile_pool(name='presp', bufs=1,
                                                   space='PSUM'))

            pools = dict(att=att, mlp=mlpp, lnp=lnp, stat=stat, yscp=yscp,
                         pA=pA,
                         ones16=ones16, ones8d=ones8d, eps128=eps128[:],
                         diag32k=diag32k, diag1=diag1, w1p=w1p, w2p=w2p)

            def tapfn(nm, t):
                if nm not in taps:
                    return
                d = nc.dram_tensor('tap_' + nm, list(t.shape),
                                   t.dtype, kind="ExternalOutput")
                nc.sync.dma_start(d[:], t[:])
            pools['tapfn'] = tapfn

            qs = [nc.sync, nc.scalar]

            def load_attw(pfx, l):
                wt = {}
                for i, w in enumerate(('wq8', 'wk8', 'wv8', 'wo8')):
                    t = watp.tile([P, KO, DE], F8, tag=f'w_{w}')
                    qs[i % 2].dma_start(t[:], din[f'{pfx}_{w}'][l])
                    wt[w] = t
                return wt

            def layer_tail(eloc8, tagf, gather):
                if gather:
                    full8 = stream.tile([P, KO, L], F8, tag=tagf)
                    _allgather_pair(nc, tc, pools, eloc8, full8, agin, agout,
                                    tagf)
                    return full8
                return None

            # ======== encoder ========
            e16 = stream.tile([P, KO, LL], F16, tag='e16_a')
            nc.sync.dma_start(e16[:], z0l16[:])
            e8 = stream.tile([P, KO, LL], F8, tag='e8_a')
            nc.scalar.dma_start(e8[:], z0l8[:])
            Zfull = stream.tile([P, KO, L], F8, tag='Zfull')
            nc.sync.dma_start(Zfull[:], z0f8[:])

            pres = presp.tile([P, KO, LL], F32, tag='pres')
            for l in range(LENC):
                wt = load_attw('enc', l)
                _attn(nc, tc, pools, pres, e16, e8, Zfull,
                      wt['wq8'], wt['wk8'], wt['wv8'], wt['wo8'],
                      None, f'e{l}a')
                e16 = stream.tile([P, KO, LL], F16, tag='e16_b')
                e8 = stream.tile([P, KO, LL], F8, tag='e8_b')
                _ln(nc, tc, pools, pres, 1.0 / SRES, e16, e8, f'e{l}ln1')
                _mlp(nc, tc, pools, pres, e16, din['enc_w1T'][l],
                     din['enc_w2T'][l], qs, f'e{l}m')
                e16 = stream.tile([P, KO, LL], F16, tag='e16_a')
                e8 = stream.tile([P, KO, LL], F8, tag='e8_a')
                _ln(nc, tc, pools, pres, 1.0, e16, e8, f'e{l}ln2')
                Zfull = layer_tail(e8, 'Zfull', True)

            # ======== decoder ========
            e16 = stream.tile([P, KO, LL], F16, tag='e16_a')
            nc.sync.dma_start(e16[:], x0l16[:])
            e8 = stream.tile([P, KO, LL], F8, tag='e8_a')
            nc.scalar.dma_start(e8[:], x0l8[:])
            Xfull = stream.tile([P, KO, L], F8, tag='Xfull')
            nc.sync.dma_start(Xfull[:], x0f8[:])

            for l in range(LDEC):
                wt = load_attw('dec', l)
                _attn(nc, tc, pools, pres, e16, e8, Xfull,
                      wt['wq8'], wt['wk8'], wt['wv8'], wt['wo8'],
                      msk8, f'd{l}s')
                e16 = stream.tile([P, KO, LL], F16, tag='e16_b')
                e8 = stream.tile([P, KO, LL], F8, tag='e8_b')
                _ln(nc, tc, pools, pres, 1.0 / SRES, e16, e8, f'd{l}ln1')
                _attn(nc, tc, pools, pres, e16, e8, Zfull,
                      wt['wq8'], wt['wk8'], wt['wv8'], wt['wo8'],
                      None, f'd{l}c')
                e16 = stream.tile([P, KO, LL], F16, tag='e16_a')
                e8 = stream.tile([P, KO, LL], F8, tag='e8_a')
                _ln(nc, tc, pools, pres, 1.0 / SRES, e16, e8, f'd{l}ln2')
                _mlp(nc, tc, pools, pres, e16, din['dec_w1T'][l],
                     din['dec_w2T'][l], qs, f'd{l}m')
                if l < LDEC - 1:
                    e16 = stream.tile([P, KO, LL], F16, tag='e16_b')
                    e8 = stream.tile([P, KO, LL], F8, tag='e8_b')
                    _ln(nc, tc, pools, pres, 1.0, e16, e8, f'd{l}ln3')
                    Xfull = layer_tail(e8, 'Xfull', True)
                else:
                    _ln(nc, tc, pools, pres, 1.0, xu, None, f'd{l}ln3')

        # ========== unembed phase (position-sharded, no collectives) ========
        with contextlib.ExitStack() as ctx:
            usb = ctx.enter_context(tc.tile_pool(name='usb', bufs=1))
            wup = ctx.enter_context(tc.tile_pool(name='wup', bufs=4))
            up = ctx.enter_context(tc.tile_pool(name='up', bufs=6,
                                                space='PSUM'))
            expu = [usb.tile([P, UNC, UC], F16, tag=f'expu{pt}',
                             name=f'expu{pt}')
                    for pt in range(2)]
            dacc = usb.tile([P, 2, UNC], F32, tag='dacc')
            qs = [nc.sync, nc.scalar]
            for c in range(UNC):
                wut = wup.tile([P, KO, UC], F16, tag='wut')
                qs[c % 2].dma_start(wut[:], wuT[:, :, c * UC:(c + 1) * UC])
                for pt in range(2):
                    ps = up.tile([P, UC], F32, tag='ups', name=f'ups{c}{pt}')
                    for k in range(KO):
                        nc.tensor.matmul(ps[:], xu[:, k, pt * P:(pt + 1) * P],
                                         wut[:, k, :], start=(k == 0),
                                         stop=(k == KO - 1))
                    nc.scalar.activation(expu[pt][:, c, :], ps[:], AF.Exp,
                                         accum_out=dacc[:, pt, c:c + 1])
            deno = usb.tile([P, 2], F32, tag='deno')
            nc.vector.tensor_reduce(deno[:], dacc[:], mybir.AxisListType.X,
                                    ALU.add)
            dinv = usb.tile([P, 2], F32, tag='dinv')
            nc.vector.reciprocal(dinv[:], deno[:])
            if 'deno' in taps:
                d = nc.dram_tensor('tap_deno', [P, 2], F32,
                                   kind="ExternalOutput")
                nc.sync.dma_start(d[:], deno[:])
            QN = UNC // 8
            for pt in range(2):
                for q in range(8):
                    seg = expu[pt][:, q * QN:(q + 1) * QN, :]
                    if q % 2 == 0:
                        nc.vector.tensor_tensor(
                            seg, seg,
                            dinv[:, pt:pt + 1, None].to_broadcast(
                                (P, QN, UC)),
                            ALU.mult)
                    else:
                        nc.scalar.activation(seg, seg, AF.Copy,
                                             scale=dinv[:, pt:pt + 1])
                    qs[(pt * 8 + q) % 2].dma_start(
                        outp[pt, :, q * QN * UC:(q + 1) * QN * UC], seg)

    nc.compile()
    return nc


# ----------------------------------------------------------------------------
# host-side prep
# ----------------------------------------------------------------------------

def _to_kimaj(a):
    """[K, M] -> [128, K//128, M] with K = ko*128 + ki."""
    K, M = a.shape
    return np.ascontiguousarray(
        a.reshape(K // P, P, M).transpose(1, 0, 2))


def _fp8(a):
    return np.clip(a, -240.0, 240.0).astype(mybir.dt.np(F8))


def _qk_pack(W):
    """Wq/Wk [H, 64, DE] -> [DE, 1024], head-major 64-dim packing."""
    return np.ascontiguousarray(
        W.reshape(H * DA, DE).T)               # [DE, col=h*64+d]


def prep_inputs(inputs):
    f = lambda k: np.asarray(inputs[k], dtype=np.float32)
    We, Wp, Wu = f('We'), f('Wp'), f('Wu')
    x = np.asarray(inputs['x']).astype(np.int64)
    z = np.asarray(inputs['z']).astype(np.int64)

    shared = {}
    for pfx, nl in (('enc', LENC), ('dec', LDEC)):
        Wq, Wk, Wv = f(pfx + '_Wq'), f(pfx + '_Wk'), f(pfx + '_Wv')
        Wo, W1, W2 = f(pfx + '_Wo'), f(pfx + '_W1'), f(pfx + '_W2')
        wq, wk, wv, wo, w1, w2 = [], [], [], [], [], []
        for l in range(nl):
            wq.append(_to_kimaj(_qk_pack(Wq[l]) * WS))
            wk.append(_to_kimaj(_qk_pack(Wk[l]) * WS))
            va = Wv[l].transpose(2, 0, 1).reshape(DE, H * DA) * WS
            wv.append(_to_kimaj(va))
            wo.append(_to_kimaj(Wo[l].T * WS))
            w1.append(_to_kimaj(W1[l].T))
            w2.append(_to_kimaj(W2[l].T))
        shared[f'{pfx}_wq8'] = _fp8(np.stack(wq))
        shared[f'{pfx}_wk8'] = _fp8(np.stack(wk))
        shared[f'{pfx}_wv8'] = _fp8(np.stack(wv))
        shared[f'{pfx}_wo8'] = _fp8(np.stack(wo))
        shared[f'{pfx}_w1T'] = np.stack(w1).astype(np.float16)
        shared[f'{pfx}_w2T'] = np.stack(w2).astype(np.float16)

    shared['wuT'] = _to_kimaj(Wu.T).astype(np.float16)
    shared['diag32k'] = (np.eye(P, dtype=np.float32) * SRES).astype(
        np.float16)
    shared['diag1'] = np.eye(P, dtype=np.float16)

    pos = Wp[:L]  # [512, 1024]
    in_maps = []
    for c in range(N_CORES):
        b, h = c // 2, c % 2
        m = dict(shared)
        for nm, tok in (('z0', z[b]), ('x0', x[b])):
            E0 = (We[tok] + pos).T.astype(np.float32)      # [1024, 512]
            E0k = E0.reshape(KO, P, L)                     # [ko, ki, p]
            full = np.ascontiguousarray(E0k.transpose(1, 0, 2))
            m[nm + '_full8'] = _fp8(full)
            loc = np.ascontiguousarray(
                E0k[:, :, h * LL:(h + 1) * LL].transpose(1, 0, 2))
            m[nm + '_loc16'] = loc.astype(np.float16)
            m[nm + '_loc8'] = _fp8(loc)
        kglob = np.arange(L)[:, None]
        qglob = (h * LL + np.arange(LL))[None, :]
        msk = (kglob <= qglob).astype(np.float32)          # [512, 256]
        m['mask_self8'] = _fp8(
            msk.reshape(4, P, LL).transpose(1, 0, 2))
        in_maps.append(m)
    return in_maps


def assemble(results):
    """results: per-core dicts with 'outp' [2, 128, NV] fp16."""
    out = np.empty((4, NV, L), dtype=np.float32)
    for c, r in enumerate(results):
        b, h = c // 2, c % 2
        o = r['outp'].reshape(LL, NV)                      # [pos, vocab]
        out[b, :, h * LL:(h + 1) * LL] = o.T.astype(np.float32)
    return out


def run(inputs, trace=False, taps=(), trace_kwargs=None):
    key = ('prog', tuple(sorted(taps)))
    if key not in _CACHE:
        _CACHE[key] = build_program(taps=taps)
    nc = _CACHE[key]
    in_maps = prep_inputs(inputs)
    res = run_bass_kernel_spmd(nc, in_maps, list(range(N_CORES)),
                               trace=trace, **(trace_kwargs or {}))
    return res


def kernel(**inputs):
    res = run(inputs, trace=False)
    return assemble(res.results)
